# revision 33
# baseline (speedup 1.0000x reference)
"""BiSPA (bidirectional sparse windowed attention + MLP) Trainium2 kernel.

Full inputs in, full outputs out. Internally shards across 8 NeuronCores:
core c owns output rows Ic = [24c, 24c+24) of the (192, 192, 512) grid.

Key observation: with B == S == 192 and window W == 32,
  - vertical attention for output row i is a complete 192-token sliding-window
    attention over x[i, :, :]                        (needs x rows  Ic)
  - horizontal attention for output row i is a complete 192-token
    sliding-window attention with Q from x[i, :, :] and K/V from x[:, i, :]
    (needs x columns Ic)
so each core needs x[Ic, :, :] and x[:, Ic, :] and NOTHING else -> zero
duplicated projection FLOPs, zero collectives, no halos.

Numerics: all matmul inputs bf16, fp32 PSUM accumulation, fp32 softmax exp
input, bf16 probs/ctx.

Schedule: the 12 strip-pair iterations are SOFTWARE-PIPELINED so the PE
never sits in a long low-duty attention stretch (which re-throttles the
HAM clock gate to 1.2 GHz): iteration g emits

   proj(g) [dense PE]  ->  scores(g,unit j) interleaved with
   attn-tail(g-1) [softmax-dependent small matmuls whose ACT/DVE inputs
   were produced an iteration earlier -> no round-trip stalls]  ->
   out-proj+MLP(g-1) [dense PE]

Per (branch, head-pair) scores live in one 2-bank PSUM tile (heads in
separate banks so their row-group-concurrent matmuls cannot collide),
giving ONE batched exp per unit via a strided 3D AP. The band uses an
EXACT 96/96 block split (queries [0,96) x keys [0,128); queries
[96,192) x keys [64,192)) so attn@V is 4 matmuls per (strip,
head-pair) with no overlap corrections. The softmax normalize uses
step-0-broadcast tensor_tensor (2 instructions instead of 4
tensor_scalars), and PSUM evictions are split across ACT (projections)
and DVE (ctx) to balance engine load.
"""

import dataclasses
import numpy as np
from contextlib import ExitStack

import concourse.bass as bass
import concourse.mybir as mybir
import concourse.tile as tile
from concourse import bacc
from concourse.bass_utils import run_bass_kernel_spmd
from concourse.masks import make_identity
from concourse.tile import add_dep_helper


def _chain(insts):
    """Order matmuls targeting one PSUM bank: a start=True zeroes (marks
    pending-zero) the WHOLE 2KB bank, so each bank must hold exactly one
    accumulation group and the group's matmuls must execute in program order.
    Tile won't order disjoint-region writes by itself."""
    for a, b in zip(insts, insts[1:]):
        add_dep_helper(b.ins, a.ins, sync=False, reason="psum-bank group order")


def _rep(ap, dims):
    """Replace the free dims of an AP (list of (step, nelem)); keeps the
    partition dim and offset. Used for step-0 broadcast reads."""
    return dataclasses.replace(ap, ap=[ap.ap[0], *dims])


BF = mybir.dt.bfloat16
F32 = mybir.dt.float32
AF = mybir.ActivationFunctionType
MUL = mybir.AluOpType.mult
NPBF = mybir.dt.np(BF)

E = 512
H = 8
D = 64
W = 32
S = 192
NCORE = 8
RPC = 24          # rows (strips) per core
T = RPC * S       # tokens per core per branch = 4608


def _band_masks():
    """Score mask, bf16 (128, 384): [TA 96 | TB 96] x 2 strips.

    Exact 2-block band split (W=32, S=192): queries [0,96) see only keys
    [0,128) and queries [96,192) see only keys [64,192), so each block is
    a plain band with no overlap correction.
    TA: rows = key k in [0,128), cols q in [0,96):   valid = |k-q| <= W
    TB: rows r -> key k = 64+r, cols q' -> q = 96+q': valid = |r-32-q'| <= W
    """
    k = np.arange(128)[:, None]
    qa = np.arange(96)[None, :]
    ta = (np.abs(k - qa) <= W)
    tb = (np.abs(k - 32 - qa) <= W)
    m = np.concatenate([ta, tb], axis=1).astype(np.float32)
    return np.concatenate([m, m], axis=1).astype(NPBF)


def _build_program(bias_flags):
    """Build the SPMD Bass/Tile program (same program on all 8 cores)."""
    nc = bacc.Bacc("TRN2", target_bir_lowering=False, debug=False,
                   num_devices=NCORE, num_swdge_queues=4)

    xr_t = nc.dram_tensor("xr_t", [E, T], BF, kind="ExternalInput").ap()
    xc_t = nc.dram_tensor("xc_t", [E, T], BF, kind="ExternalInput").ap()
    w_vin = nc.dram_tensor("w_vin", [E, 3 * E], BF, kind="ExternalInput").ap()
    w_hq = nc.dram_tensor("w_hq", [E, E], BF, kind="ExternalInput").ap()
    w_hkv = nc.dram_tensor("w_hkv", [E, 2 * E], BF, kind="ExternalInput").ap()
    w_vout = nc.dram_tensor("w_vout", [E, E], BF, kind="ExternalInput").ap()
    w_hout = nc.dram_tensor("w_hout", [E, E], BF, kind="ExternalInput").ap()
    w_m1 = nc.dram_tensor("w_m1", [2 * E, E], BF, kind="ExternalInput").ap()
    w_m2 = nc.dram_tensor("w_m2", [E, E], BF, kind="ExternalInput").ap()
    mask_d = nc.dram_tensor("mask", [128, 384], BF, kind="ExternalInput").ap()
    bias_d = nc.dram_tensor("biases", [128, 32], F32, kind="ExternalInput").ap()
    out_t = nc.dram_tensor("out_t", [E, T], F32, kind="ExternalOutput").ap()

    with tile.TileContext(nc) as tc, ExitStack() as ctx:
        pw = ctx.enter_context(tc.tile_pool(name="pw", bufs=1))
        psA = ctx.enter_context(tc.tile_pool(name="psA", bufs=3, space="PSUM"))
        psS = ctx.enter_context(tc.tile_pool(name="psS", bufs=1, space="PSUM"))
        psC = ctx.enter_context(tc.tile_pool(name="psC", bufs=2, space="PSUM"))
        px = ctx.enter_context(tc.tile_pool(name="px", bufs=4))
        pqk = ctx.enter_context(tc.tile_pool(name="pqk", bufs=18))
        pv = ctx.enter_context(tc.tile_pool(name="pv", bufs=18))
        ppb = ctx.enter_context(tc.tile_pool(name="ppb", bufs=3))
        ppm = ctx.enter_context(tc.tile_pool(name="ppm", bufs=18))
        pctx = ctx.enter_context(tc.tile_pool(name="pctx", bufs=4))
        pzr = ctx.enter_context(tc.tile_pool(name="pzr", bufs=4))
        pct = ctx.enter_context(tc.tile_pool(name="pct", bufs=12))
        pcomb = ctx.enter_context(tc.tile_pool(name="pcomb", bufs=10))
        phid = ctx.enter_context(tc.tile_pool(name="phid", bufs=6))
        pout = ctx.enter_context(tc.tile_pool(name="pout", bufs=6))

        # ---- persistent constants. Each weight matrix loads as ONE strided
        # DMA into a [128, ktiles, cols] tile (34 serial ~650ns DMA issues
        # delayed the first evictions by ~12us when split per k-tile); the
        # per-k-tile views are plain slices. Projection weights first. ----
        def load_group(name, dram_ap, ktiles, cols):
            t = pw.tile([128, ktiles, cols], BF, tag=name, name=name)
            nc.gpsimd.dma_start(
                t[:], dram_ap.rearrange("(k p) c -> p k c", p=128))
            return [t[:, k, :] for k in range(ktiles)]

        whq = load_group("whq", w_hq, 4, E)
        whkv = load_group("whkv", w_hkv, 4, 2 * E)
        wv = load_group("wv", w_vin, 4, 3 * E)
        msk = pw.tile([128, 384], BF, tag="msk", name="msk")
        nc.gpsimd.dma_start(msk[:], mask_d[:, :])
        bia = pw.tile([128, 32], F32, tag="bia", name="bia")
        nc.gpsimd.dma_start(bia[:], bias_d[:, :])
        wvo = load_group("wvo", w_vout, 4, E)
        who = load_group("who", w_hout, 4, E)
        wm1 = load_group("wm1", w_m1, 8, E)
        wm2 = load_group("wm2", w_m2, 4, E)
        ident = pw.tile([128, 128], BF, tag="ident")
        make_identity(nc, ident)

        # bias column map (within `bia`):
        # 0-7 v_in_b[0:1024] ftiles; 8-11 h_in_b[0:512]; 12-15 h_in_b[512:1024]
        # 16-19 h_out_eff; 20-23 v_out_eff; 24-27 mlp_b1; 28-31 mlp_b2

        NPAIR = RPC // 2

        # cross-iteration registries
        qk_t = {}     # g -> {br: [8 tiles]}
        v_t = {}      # g -> {(a, br): (va, vb)}
        pm_t = {}     # g -> {(br, p): pm tile [128, 896]}
        ct_t = {}     # g -> {br: [4 ct tiles (128, 384)]}

        def emit_stage(g):
            g0 = 2 * S * g
            xr2, xc2 = [], []
            for k in range(4):
                t = px.tile([128, 2 * S], BF, tag=f"xr{k}", name=f"xr{k}_{g}")
                nc.sync.dma_start(t[:], xr_t[128 * k:128 * (k + 1), g0:g0 + 2 * S])
                xr2.append(t)
                t = px.tile([128, 2 * S], BF, tag=f"xc{k}", name=f"xc{k}_{g}")
                nc.sync.dma_start(t[:], xc_t[128 * k:128 * (k + 1), g0:g0 + 2 * S])
                xc2.append(t)
            return xr2, xc2

        def emit_qk_group(g, br, j, xr2, xc2):
            ps = psA.tile([128, 384], F32, tag="proj", name=f"qkps_{g}",
                          padded_shape=[128, 512])
            for k in range(4):
                if br == "v":
                    lhsT = wv[k][:, 128 * j:128 * (j + 1)]
                    rhs = xr2[k][:]
                elif j < 4:   # h Q
                    lhsT = whq[k][:, 128 * j:128 * (j + 1)]
                    rhs = xr2[k][:]
                else:         # h K
                    lhsT = whkv[k][:, 128 * (j - 4):128 * (j - 3)]
                    rhs = xc2[k][:]
                nc.tensor.matmul(ps[:], lhsT=lhsT, rhs=rhs,
                                 start=(k == 0), stop=(k == 3))
            bcol = j if br == "v" else (8 + j)
            dst = pqk.tile([128, 384], BF, tag="qk", name=f"qk_{g}")
            nc.scalar.activation(dst[:], ps[:], AF.Identity,
                                 bias=bia[:, bcol:bcol + 1])
            qk_t[g].setdefault(br, {})[j] = dst

        def emit_v_unit(g, a, br, xr2, xc2):
            s0 = S * a
            xin = xr2 if br == "v" else xc2
            vcols = slice(1024, 1536) if br == "v" else slice(512, 1024)
            vw = wv if br == "v" else whkv
            vps_a = psA.tile([128, 512], F32, tag="proj", name=f"vpsa_{g}")
            vps_b = psA.tile([128, 512], F32, tag="proj", name=f"vpsb_{g}")
            for k in range(4):
                nc.tensor.matmul(vps_a[:], lhsT=xin[k][:, s0:s0 + 128],
                                 rhs=vw[k][:, vcols],
                                 start=(k == 0), stop=(k == 3))
            for k in range(4):
                nc.tensor.matmul(vps_b[:], lhsT=xin[k][:, s0 + 64:s0 + 192],
                                 rhs=vw[k][:, vcols],
                                 start=(k == 0), stop=(k == 3))
            va = pv.tile([128, 8, 65], BF, tag="vp", name=f"va_{g}")
            vb = pv.tile([128, 8, 65], BF, tag="vp", name=f"vb_{g}")
            nc.vector.tensor_copy(
                va[:, :, 0:64],
                vps_a[:].rearrange("p (h c) -> p h c", c=64))
            nc.vector.tensor_copy(
                vb[:, :, 0:64],
                vps_b[:].rearrange("p (h c) -> p h c", c=64))
            nc.vector.memset(va[:, :, 64:65], 1.0)
            nc.vector.memset(vb[:, :, 64:65], 1.0)
            v_t[g][(a, br)] = (va, vb)

        def emit_scores(g, br, p):
            """Scores + exp + mask for both strips of (br, p) of pair g.
            One 2-bank PSUM tile: head h2 in bank h2 at cols
            512*h2 + 192*a + [0:192] ([TA 96 | TB 96])."""
            qk = qk_t[g][br]
            sp = psS.tile([128, 1024], F32, tag="sc", name=f"sc_{g}_{br}_{p}")
            for h2 in range(2):
                d0 = 64 * h2
                for a in range(2):
                    s0 = S * a
                    QT = qk[p][:, s0:s0 + S]
                    KT = qk[4 + p][:, s0:s0 + S]
                    cb = 512 * h2 + 192 * a
                    nc.tensor.matmul(sp[:, cb:cb + 96],
                                     lhsT=KT[d0:d0 + 64, 0:128],
                                     rhs=QT[d0:d0 + 64, 0:96],
                                     start=True, stop=True)
                    nc.tensor.matmul(sp[:, cb + 96:cb + 192],
                                     lhsT=KT[d0:d0 + 64, 64:192],
                                     rhs=QT[d0:d0 + 64, 96:192],
                                     start=True, stop=True)
            pb = ppb.tile([128, 768], BF, tag="pb", name=f"pb_{g}")
            nc.scalar.activation(
                pb[:].rearrange("p (b c) -> p b c", c=384),
                sp[:].rearrange("p (b c) -> p b c", c=512)[:, :, 0:384],
                AF.Exp, scale=0.125)
            pm = ppm.tile([128, 768], BF, tag="pm", name=f"pm_{g}_{br}_{p}")
            # band-mask multiply (step-0 broadcast over the head dim; mask
            # cols are [strip a | strip b] and identical per head)
            nc.vector.tensor_tensor(
                pm[:].rearrange("p (b c) -> p b c", c=384),
                pb[:].rearrange("p (b c) -> p b c", c=384),
                _rep(msk[:, 0:384], [(0, 2), (1, 384)]), op=MUL)
            pm_t[g].setdefault(br, {})[p] = pm

        def emit_attn_tail(g, a, br, p):
            """attn@V + normalize + transpose + evict for (strip a, br, p)
            of pair g, TOKEN-major (q on partitions; ones-column in V
            accumulates the softmax denominator Z so the normalize is a
            cheap per-partition scalar multiply). A feature-major variant
            (no transposes) was tried and measured SLOWER: 1/Z then lives
            on 1-2 partitions, and DVE reciprocal/broadcast cost scales
            with elements PER LANE (~1.3us per 192-elem row)."""
            pm = pm_t[g][br][p]
            va, vb = v_t[g][(a, br)]
            s0 = S * a
            cp = psC.tile([128, 512], F32, tag="cx", name=f"cx_{g}")
            mms = []
            for h2 in range(2):
                h = 2 * p + h2
                cb = 130 * h2
                ta = 384 * h2 + 192 * a
                tb = ta + 96
                # q in [0,96): keys [0:128) from TA. start=True on the
                # first matmul clears has_written for the full bank-row of
                # partitions 0:96 (per-partition-range, HW-verified), which
                # covers every later write in this group.
                mms.append(nc.tensor.matmul(
                    cp[0:96, cb:cb + 65], lhsT=pm[:, ta:ta + 96],
                    rhs=va[:, h:h + 1, :], start=(h2 == 0),
                    stop=False, skip_group_check=True))
                # q in [96,192): keys [64:192) from TB
                mms.append(nc.tensor.matmul(
                    cp[0:96, cb + 65:cb + 130],
                    lhsT=pm[:, tb:tb + 96],
                    rhs=vb[:, h:h + 1, :],
                    start=False, stop=(h2 == 1),
                    skip_group_check=True))
            _chain(mms)

            # normalize by 1/Z. Z columns of cp: 64 (h0 q1), 129 (h0 q2),
            # 194 (h1 q1), 259 (h1 q2) -> strided recip into zr[:, 0:4]
            # ordered (h0q1, h0q2, h1q1, h1q2); all on partitions 0:96.
            zr = pzr.tile([128, 4, 1], F32, tag="zr", name=f"zr_{g}")
            ctxn = pctx.tile([128, 256], BF, tag="ctxn", name=f"ctxn_{g}")
            reads = [
                nc.vector.reciprocal(
                    zr[0:96, :, :], _rep(cp[0:96, 64:65], [(65, 4), (1, 1)])),
                # ctxn[0:96, 0:128] = [h0 q1 | h1 q1]: cp cols (0,130)+64
                # scaled by zr rows (0, 2)
                nc.vector.tensor_tensor(
                    ctxn[0:96, 0:128].rearrange("p (b c) -> p b c", c=64),
                    _rep(cp[0:96, 0:1], [(130, 2), (1, 64)]),
                    _rep(zr[0:96, 0:1, 0:1], [(2, 2), (0, 64)]), op=MUL),
                # ctxn[0:96, 128:256] = [h0 q2 | h1 q2]: cp cols (65,195)+64
                # scaled by zr rows (1, 3)
                nc.vector.tensor_tensor(
                    ctxn[0:96, 128:256].rearrange("p (b c) -> p b c", c=64),
                    _rep(cp[0:96, 65:66], [(130, 2), (1, 64)]),
                    _rep(zr[0:96, 1:2, 0:1], [(2, 2), (0, 64)]), op=MUL),
            ]
            # cp reads must wait for the accumulation group to close
            # (same-bank PE-write + DVE-read is a HW fault)
            for r in reads:
                add_dep_helper(r.ins, mms[-1].ins, sync=True,
                               reason="psum read after group close")

            ct_p = ct_t[g][br][p]
            ctp = psC.tile([128, S], BF, tag="cxT", bufs=1, name=f"ctp_{g}")
            nc.tensor.transpose(ctp[:, 0:96], ctxn[0:96, 0:128],
                                ident[0:96, 0:96])
            nc.tensor.transpose(ctp[:, 96:192], ctxn[0:96, 128:256],
                                ident[0:96, 0:96])
            nc.vector.tensor_copy(ct_p[:, s0:s0 + S], ctp[:])

        def emit_outproj_tile(g, br, j, comb):
            wout = wvo if br == "v" else who
            bb0 = 20 if br == "v" else 16
            ps = psA.tile([128, 384], F32, tag="proj", name=f"ops_{g}",
                          padded_shape=[128, 512])
            for k in range(4):
                nc.tensor.matmul(ps[:],
                                 lhsT=wout[k][:, 128 * j:128 * (j + 1)],
                                 rhs=ct_t[g][br][k][:],
                                 start=(k == 0), stop=(k == 3))
            idx = j + (4 if br == "v" else 0)
            dst = pcomb.tile([128, 384], BF, tag="comb", name=f"comb_{g}")
            nc.scalar.activation(dst[:], ps[:], AF.Identity,
                                 bias=bia[:, bb0 + j:bb0 + j + 1])
            comb[idx] = dst

        def emit_mlp(g, comb):
            g0 = 2 * S * g
            hid = []
            for j in range(4):
                ps = psA.tile([128, 384], F32, tag="proj", name=f"hps_{g}",
                              padded_shape=[128, 512])
                for k in range(8):
                    nc.tensor.matmul(ps[:],
                                     lhsT=wm1[k][:, 128 * j:128 * (j + 1)],
                                     rhs=comb[k][:],
                                     start=(k == 0), stop=(k == 7))
                dst = phid.tile([128, 384], BF, tag="hid", name=f"hid_{g}")
                nc.scalar.activation(dst[:], ps[:], AF.Relu,
                                     bias=bia[:, 24 + j:24 + j + 1])
                hid.append(dst)
            for j in range(4):
                ps = psA.tile([128, 384], F32, tag="proj", name=f"mps_{g}",
                              padded_shape=[128, 512])
                for k in range(4):
                    nc.tensor.matmul(ps[:],
                                     lhsT=wm2[k][:, 128 * j:128 * (j + 1)],
                                     rhs=hid[k][:],
                                     start=(k == 0), stop=(k == 3))
                osb = pout.tile([128, 384], F32, tag="o", name=f"o_{g}")
                nc.scalar.activation(osb[:], ps[:], AF.Identity,
                                     bias=bia[:, 28 + j:28 + j + 1])
                nc.sync.dma_start(out_t[128 * j:128 * (j + 1), g0:g0 + 2 * S],
                                  osb[:])

        TAILS = [(a, br, p) for a in range(2) for br in ("h", "v")
                 for p in range(4)]

        for g in range(NPAIR + 1):
            if g > 0:
                ct_t[g - 1] = {br: [pct.tile([128, 2 * S], BF, tag="ct",
                                             name=f"ct_{br}_{g - 1}_{p}")
                                    for p in range(4)] for br in ("h", "v")}
            comb = [None] * 8
            if g < NPAIR:
                pm_t[g] = {}
                qk_t[g] = {}
                v_t[g] = {}
                xr2, xc2 = emit_stage(g)
                # fine-grained interleave: sparse attention units are woven
                # between dense projection groups so the PE array duty never
                # drops low enough for the HAM clock gate to re-throttle.
                ti = iter(range(16))

                def tails2():
                    if g > 0:
                        emit_attn_tail(g - 1, *TAILS[next(ti)])
                        emit_attn_tail(g - 1, *TAILS[next(ti)])

                if g == 0:
                    # pair 0 has no prior-pair tails to fill the exp
                    # round-trip between score units; interleave the
                    # v-branch projection groups there instead.
                    for j in range(5):
                        emit_qk_group(g, "h", j, xr2, xc2)
                    emit_v_unit(g, 0, "h", xr2, xc2)
                    for p in range(4):
                        emit_scores(g, "h", p)
                        if p < 3:
                            emit_qk_group(g, "h", 5 + p, xr2, xc2)
                            emit_qk_group(g, "v", p, xr2, xc2)
                    emit_v_unit(g, 1, "h", xr2, xc2)
                    emit_qk_group(g, "v", 3, xr2, xc2)
                    emit_qk_group(g, "v", 4, xr2, xc2)
                    emit_v_unit(g, 0, "v", xr2, xc2)
                    for p in range(4):
                        emit_scores(g, "v", p)
                        if p < 3:
                            emit_qk_group(g, "v", 5 + p, xr2, xc2)
                    emit_v_unit(g, 1, "v", xr2, xc2)
                else:
                    for br_i, br in enumerate(("h", "v")):
                        for j in range(5):
                            emit_qk_group(g, br, j, xr2, xc2)
                        emit_v_unit(g, 0, br, xr2, xc2)
                        for p in range(4):
                            if p > 0:
                                emit_qk_group(g, br, 4 + p, xr2, xc2)
                            emit_scores(g, br, p)
                            tails2()
                        emit_v_unit(g, 1, br, xr2, xc2)
                if g > 0:
                    for br in ("h", "v"):
                        for j in range(4):
                            emit_outproj_tile(g - 1, br, j, comb)
            else:
                # drain: interleave the last pair's out-proj tiles with its
                # remaining tails so the PE keeps some dense work
                ht = [t_ for t_ in TAILS if t_[1] == "h"]
                vt = [t_ for t_ in TAILS if t_[1] == "v"]
                for t_ in ht:
                    emit_attn_tail(g - 1, *t_)
                for j in range(4):
                    emit_outproj_tile(g - 1, "h", j, comb)
                    emit_attn_tail(g - 1, *vt[2 * j])
                    emit_attn_tail(g - 1, *vt[2 * j + 1])
                for j in range(4):
                    emit_outproj_tile(g - 1, "v", j, comb)
            if g > 0:
                emit_mlp(g - 1, comb)
                # drop references no longer needed
                for reg in (qk_t, v_t, pm_t, ct_t):
                    reg.pop(g - 2, None)
    nc.finalize()
    return nc


_CACHE = {}


def _get_program(bias_flags):
    key = tuple(bias_flags)
    if key not in _CACHE:
        _CACHE[key] = _build_program(key)
    return _CACHE[key]


def _col(b):
    """bias vector (128*n,) -> (128, n) column-pack, fortran-ish layout."""
    return np.ascontiguousarray(b.reshape(-1, 128).T.astype(np.float32))


def kernel(hidden_states, h_in_w, h_in_b, h_out_w, h_out_b,
           v_in_w, v_in_b, v_out_w, v_out_b,
           mlp_w1, mlp_b1, mlp_w2, mlp_b2):
    x = np.asarray(hidden_states, dtype=np.float32)
    h_in_w = np.asarray(h_in_w, np.float32)
    h_in_b = np.asarray(h_in_b, np.float32)
    h_out_w = np.asarray(h_out_w, np.float32)
    h_out_b = np.asarray(h_out_b, np.float32)
    v_in_w = np.asarray(v_in_w, np.float32)
    v_in_b = np.asarray(v_in_b, np.float32)
    v_out_w = np.asarray(v_out_w, np.float32)
    v_out_b = np.asarray(v_out_b, np.float32)
    mlp_w1 = np.asarray(mlp_w1, np.float32)
    mlp_b1 = np.asarray(mlp_b1, np.float32)
    mlp_w2 = np.asarray(mlp_w2, np.float32)
    mlp_b2 = np.asarray(mlp_b2, np.float32)

    # V biases act as a constant shift of ctx (softmax weights sum to 1),
    # so fold them through the out-projections.
    h_out_eff = h_out_b + h_out_w @ h_in_b[2 * E:3 * E]
    v_out_eff = v_out_b + v_out_w @ v_in_b[2 * E:3 * E]

    bias_flags = (
        bool(np.any(v_in_b[0:2 * E])), bool(np.any(h_in_b[0:E])),
        bool(np.any(h_in_b[E:2 * E])), bool(np.any(h_out_eff)),
        bool(np.any(v_out_eff)), bool(np.any(mlp_b1)), bool(np.any(mlp_b2)),
    )
    nc = _get_program(bias_flags)

    biases = np.zeros((128, 32), np.float32)
    biases[:, 0:8] = _col(v_in_b[0:2 * E])
    biases[:, 8:16] = _col(h_in_b[0:2 * E])
    biases[:, 16:20] = _col(h_out_eff)
    biases[:, 20:24] = _col(v_out_eff)
    biases[:, 24:28] = _col(mlp_b1)
    biases[:, 28:32] = _col(mlp_b2)

    shared = {
        "w_vin": np.ascontiguousarray(v_in_w.T).astype(NPBF),
        "w_hq": np.ascontiguousarray(h_in_w[0:E].T).astype(NPBF),
        "w_hkv": np.ascontiguousarray(h_in_w[E:3 * E].T).astype(NPBF),
        "w_vout": np.ascontiguousarray(v_out_w.T).astype(NPBF),
        "w_hout": np.ascontiguousarray(h_out_w.T).astype(NPBF),
        "w_m1": np.ascontiguousarray(mlp_w1.T).astype(NPBF),
        "w_m2": np.ascontiguousarray(mlp_w2.T).astype(NPBF),
        "mask": _band_masks(),
        "biases": biases,
    }

    in_maps = []
    for c in range(NCORE):
        rows = x[RPC * c:RPC * (c + 1)]                      # (24, 192, 512)
        cols = x[:, RPC * c:RPC * (c + 1)].transpose(1, 0, 2)  # (24, 192, 512)
        m = dict(shared)
        m["xr_t"] = np.ascontiguousarray(rows.reshape(T, E).T).astype(NPBF)
        m["xc_t"] = np.ascontiguousarray(cols.reshape(T, E).T).astype(NPBF)
        in_maps.append(m)

    global _LAST_IN_MAPS
    _LAST_IN_MAPS = in_maps
    res = run_bass_kernel_spmd(nc, in_maps, core_ids=list(range(NCORE)))

    out = np.empty((S, S, E), np.float32)
    for c in range(NCORE):
        out[RPC * c:RPC * (c + 1)] = res.results[c]["out_t"].T.reshape(RPC, S, E)
    return out
